# revision 4
# baseline (speedup 1.0000x reference)
"""GATv2Conv kernel for 8 Trainium2 NeuronCores.

Strategy: destination-node sharding. Edges (with self loops) are sorted by
destination row and split into 8 contiguous node ranges with balanced edge
counts. Per core the host ships a per-edge stream s_e = x[row_e] + x[col_e]
(transposed, channels on partitions). The device computes, per 128-edge tile:
  E = s @ W            (PE, two layouts: ch-on-part and edge-on-part)
  e_act = lrelu(E)     (ACT)
  alpha_T = e_act^T @ attmask   (PE)  -> ea = exp(alpha_T) (ACT)
  wmsg = E_T * ea      (DVE, broadcast over channels)
  acc += sel^T @ [wmsg | ea]    (PE, selection matrix from is_equal vs iota)
Per 128-node stripe the accumulated numerator is corrected by
  num = acc[:, :64] - h_i * den   (since E = h_i + h_j)
and divided by den = acc[:, 64:68]. h_i = x_slice @ W computed on device.
No softmax max-subtraction: alpha is O(10) so exp is safe in fp32.
"""
import os
import sys
import types

sys.path.insert(0, "/opt/trn_rl_repo")

import numpy as np

N = 100000
E_RAW = 1600000
IN = 128
H, C = 4, 16
HC = H * C
N_CORES = 8
P = 128
QUAD = 4  # tiles per quad (batched elementwise/matmul group)

_CACHE = {}
LAST_EXEC_NS = None


def _install_axon_ntff_shim():
    if "antenv.axon_hooks" in sys.modules:
        return
    try:
        sys.path.insert(0, "/root/.axon_site/trn_agent_boot")
        import trn_boot  # type: ignore

        hook = trn_boot._ntff_profile_via_ctypes("/opt/axon/libaxon_pjrt.so")
        mod = types.ModuleType("antenv.axon_hooks")
        _state = {"hook": hook}
        mod.set_axon_ntff_profile_hook = lambda h: _state.__setitem__("hook", h)
        mod.get_axon_ntff_profile_hook = lambda: _state["hook"]
        sys.modules["antenv.axon_hooks"] = mod
        import antenv

        antenv.axon_hooks = mod
    except Exception:
        pass


def _build_program(S, TPS):
    from concourse import bass, bacc, mybir
    import concourse.tile as tile

    key = (S, TPS)
    if key in _CACHE:
        return _CACHE[key]

    T = S * TPS
    f32 = mybir.dt.float32
    nc = bacc.Bacc("TRN2", target_bir_lowering=False, debug=False,
                   num_devices=N_CORES)
    sT = nc.dram_tensor("sT", [P, T * P], f32, kind="ExternalInput")
    xsT = nc.dram_tensor("xsT", [P, S * P], f32, kind="ExternalInput")
    rowrel = nc.dram_tensor("rowrel", [P, T], f32, kind="ExternalInput")
    Wt = nc.dram_tensor("W", [IN, HC], f32, kind="ExternalInput")
    attm = nc.dram_tensor("attm", [HC, H], f32, kind="ExternalInput")
    out_d = nc.dram_tensor("out", [S * P, HC], f32, kind="ExternalOutput")

    NQ = TPS // QUAD  # quads per stripe
    assert TPS % QUAD == 0

    with tile.TileContext(nc) as tc:
        with (
            tc.tile_pool(name="const", bufs=1) as constp,
            tc.tile_pool(name="stream", bufs=2) as streamp,
            tc.tile_pool(name="work", bufs=3) as workp,
            tc.tile_pool(name="ep", bufs=2) as epp,
            tc.tile_pool(name="ps_e", bufs=2, space="PSUM") as ps_e,
            tc.tile_pool(name="ps_q", bufs=2, space="PSUM") as ps_q,
            tc.tile_pool(name="ps_at", bufs=1, space="PSUM") as ps_at,
            tc.tile_pool(name="ps_acc", bufs=2, space="PSUM") as ps_acc,
            tc.tile_pool(name="ps_h", bufs=1, space="PSUM") as ps_h,
        ):
            W_sb = constp.tile([IN, HC], f32, tag="w")
            nc.sync.dma_start(W_sb[:], Wt[:])
            attm_sb = constp.tile([HC, H], f32, tag="attm")
            nc.sync.dma_start(attm_sb[:], attm[:])
            rr_sb = constp.tile([P, T], f32, tag="rr")
            nc.sync.dma_start(rr_sb[:], rowrel[:])
            iota_i = constp.tile([P, P], mybir.dt.int32, tag="ioti")
            nc.gpsimd.iota(iota_i[:], pattern=[[1, P]], base=0,
                           channel_multiplier=0)
            iota_f = constp.tile([P, P], f32, tag="iotf")
            nc.vector.tensor_copy(iota_f[:], iota_i[:])

            for s in range(S):
                stream_sb = streamp.tile([P, TPS * P], f32, tag="stream")
                nc.sync.dma_start(stream_sb[:], sT[:, s * TPS * P:(s + 1) * TPS * P])
                acc_ps = ps_acc.tile([P, HC + H], f32, tag="acc")
                for q in range(NQ):
                    e_ps = ps_e.tile([HC, QUAD * P], f32, tag="e")
                    # E (ch-on-part) for the whole quad
                    nc.tensor.matmul(
                        out=e_ps[:],
                        lhsT=W_sb[:],
                        rhs=stream_sb[:, q * QUAD * P:(q + 1) * QUAD * P],
                        start=True, stop=True)
                    # lrelu(x) with slope 0.2 = 0.4*(1.5x + |x|); the 0.4 is
                    # folded into attmask on the host.
                    e_abs = workp.tile([HC, QUAD * P], f32, tag="eabs")
                    nc.scalar.activation(
                        out=e_abs[:], in_=e_ps[:],
                        func=mybir.ActivationFunctionType.Abs)
                    e_act = workp.tile([HC, QUAD * P], f32, tag="eact")
                    nc.vector.scalar_tensor_tensor(
                        out=e_act[:], in0=e_ps[:], scalar=1.5,
                        in1=e_abs[:], op0=mybir.AluOpType.mult,
                        op1=mybir.AluOpType.add)
                    q_ps = ps_q.tile([P, QUAD, HC + H], f32, tag="q")
                    at_ps = ps_at.tile([P, QUAD, H], f32, tag="at")
                    wmsg = workp.tile([P, QUAD, HC + H], f32, tag="wmsg")
                    for i in range(QUAD):
                        t = q * QUAD + i
                        # E_T (edge-on-part)
                        nc.tensor.matmul(
                            out=q_ps[:, i, 0:HC],
                            lhsT=stream_sb[:, t * P:(t + 1) * P],
                            rhs=W_sb[:], start=True, stop=True)
                        # alpha_T
                        nc.tensor.matmul(
                            out=at_ps[:, i, :],
                            lhsT=e_act[:, i * P:(i + 1) * P],
                            rhs=attm_sb[:], start=True, stop=True)
                    # ea = exp(alpha) for the quad, written into wmsg tail
                    nc.scalar.activation(
                        out=wmsg[:, :, HC:HC + H], in_=at_ps[:],
                        func=mybir.ActivationFunctionType.Exp)
                    # wmsg head = E_T * ea (broadcast over 16 channels)
                    nc.vector.tensor_tensor(
                        out=wmsg[:, :, 0:HC].rearrange("p q (h c) -> p q h c", h=H),
                        in0=q_ps[:, :, 0:HC].rearrange("p q (h c) -> p q h c", h=H),
                        in1=wmsg[:, :, HC:HC + H].to_broadcast([P, QUAD, H, C]),
                        op=mybir.AluOpType.mult)
                    for i in range(QUAD):
                        t = q * QUAD + i
                        sel = workp.tile([P, P], f32, tag="sel")
                        nc.vector.tensor_tensor(
                            out=sel[:],
                            in0=rr_sb[:, s * TPS + t:s * TPS + t + 1].to_broadcast([P, P]),
                            in1=iota_f[:],
                            op=mybir.AluOpType.is_equal)
                        nc.tensor.matmul(
                            out=acc_ps[:],
                            lhsT=sel[:],
                            rhs=wmsg[:, i, :],
                            start=(q == 0 and i == 0),
                            stop=(q == NQ - 1 and i == QUAD - 1))
                # epilogue
                xs_sb = epp.tile([P, P], f32, tag="xs")
                nc.sync.dma_start(xs_sb[:], xsT[:, s * P:(s + 1) * P])
                h_ps = ps_h.tile([P, HC], f32, tag="h")
                nc.tensor.matmul(
                    out=h_ps[:], lhsT=xs_sb[:],
                    rhs=W_sb[:], start=True, stop=True)
                acc_sb = epp.tile([P, HC + H], f32, tag="accsb")
                nc.scalar.activation(out=acc_sb[:], in_=acc_ps[:],
                                     func=mybir.ActivationFunctionType.Copy)
                h_sb = epp.tile([P, HC], f32, tag="hsb")
                nc.scalar.activation(out=h_sb[:], in_=h_ps[:],
                                     func=mybir.ActivationFunctionType.Copy)
                rec = epp.tile([P, H], f32, tag="rec")
                nc.vector.reciprocal(rec[:], acc_sb[:, HC:HC + H])
                tmp = epp.tile([P, HC], f32, tag="tmp")
                # tmp = h_i * den
                nc.vector.tensor_tensor(
                    out=tmp[:].rearrange("p (h c) -> p h c", h=H),
                    in0=h_sb[:].rearrange("p (h c) -> p h c", h=H),
                    in1=acc_sb[:, HC:HC + H].to_broadcast([P, H, C]),
                    op=mybir.AluOpType.mult)
                # tmp = acc_num - tmp
                nc.vector.tensor_tensor(
                    out=tmp[:], in0=acc_sb[:, 0:HC], in1=tmp[:],
                    op=mybir.AluOpType.subtract)
                out_sb = epp.tile([P, HC], f32, tag="outsb")
                nc.vector.tensor_tensor(
                    out=out_sb[:].rearrange("p (h c) -> p h c", h=H),
                    in0=tmp[:].rearrange("p (h c) -> p h c", h=H),
                    in1=rec[:].to_broadcast([P, H, C]),
                    op=mybir.AluOpType.mult)
                nc.sync.dma_start(out_d[s * P:(s + 1) * P, :], out_sb[:])
    nc.compile()
    _CACHE[key] = nc
    return nc


def _prep(x, edge_index):
    """Returns per-core input maps + (S, TPS, core node offsets/counts)."""
    x = np.asarray(x, dtype=np.float32)
    rows = np.concatenate([np.asarray(edge_index[0]), np.arange(N, dtype=np.int64)])
    cols = np.concatenate([np.asarray(edge_index[1]), np.arange(N, dtype=np.int64)])
    order = np.argsort(rows, kind="stable")
    rows = rows[order]
    cols = cols[order]
    Etot = rows.shape[0]

    n_stripes = (N + P - 1) // P  # 782
    # edges per stripe via searchsorted on sorted rows
    stripe_starts = np.searchsorted(rows, np.arange(n_stripes) * P)
    stripe_ends = np.searchsorted(rows, np.minimum((np.arange(n_stripes) + 1) * P, N))
    stripe_cnt = stripe_ends - stripe_starts
    cum = np.cumsum(stripe_cnt)
    # balanced contiguous stripe ranges per core
    bounds = [0]
    for k in range(1, N_CORES):
        bounds.append(int(np.searchsorted(cum, Etot * k / N_CORES)))
    bounds.append(n_stripes)
    S = max(bounds[k + 1] - bounds[k] for k in range(N_CORES))
    TPS = 0
    for s in range(n_stripes):
        TPS = max(TPS, (int(stripe_cnt[s]) + P - 1) // P)
    TPS = ((TPS + QUAD - 1) // QUAD) * QUAD
    T = S * TPS

    x_ext = np.vstack([x, np.zeros((1, IN), np.float32)])  # pad row -> zeros
    ins = []
    meta = []
    for k in range(N_CORES):
        s0, s1 = bounds[k], bounds[k + 1]
        node0 = s0 * P
        n_nodes = min(s1 * P, N) - node0
        slot_rows = np.full(T * P, N, dtype=np.int64)   # pad -> zero row
        slot_cols = np.full(T * P, N, dtype=np.int64)
        rowrel = np.full(T * P, 999.0, dtype=np.float32)
        for si in range(s1 - s0):
            s = s0 + si
            e0, e1 = int(stripe_starts[s]), int(stripe_ends[s])
            cnt = e1 - e0
            base = si * TPS * P
            slot_rows[base:base + cnt] = rows[e0:e1]
            slot_cols[base:base + cnt] = cols[e0:e1]
            rowrel[base:base + cnt] = (rows[e0:e1] - (node0 + si * P)).astype(np.float32)
        sA = x_ext[slot_rows]
        sA += x_ext[slot_cols]
        sT = np.ascontiguousarray(sA.T)
        del sA
        # x slice (transposed) for h_i
        sl = np.arange(node0, node0 + S * P)
        sl = np.where(sl < N, sl, N)
        xsT = np.ascontiguousarray(x_ext[sl].T)
        rr = np.ascontiguousarray(rowrel.reshape(T, P).T)
        ins.append({"sT": sT, "xsT": xsT, "rowrel": rr})
        meta.append((node0, n_nodes))
    return ins, meta, S, TPS


def kernel(x, edge_index, W, att, bias):
    global LAST_EXEC_NS
    _install_axon_ntff_shim()
    from concourse.bass_utils import run_bass_kernel_spmd

    W = np.asarray(W, dtype=np.float32)
    att = np.asarray(att, dtype=np.float32)
    bias = np.asarray(bias, dtype=np.float32)

    ins, meta, S, TPS = _prep(x, edge_index)
    attm = np.zeros((HC, H), np.float32)
    attf = att[0].reshape(HC)  # (h, c) flattened, h-major
    for hc in range(HC):
        # 0.4 factor from lrelu decomposition 0.2-slope = 0.4*(1.5x + |x|)
        attm[hc, hc // C] = 0.4 * attf[hc]
    for m in ins:
        m["W"] = W
        m["attm"] = attm

    nc = _build_program(S, TPS)
    trace = os.environ.get("KERNEL_TRACE", "1") == "1"
    try:
        res = run_bass_kernel_spmd(nc, ins, core_ids=list(range(N_CORES)),
                                   trace=trace)
    except Exception:
        if not trace:
            raise
        res = run_bass_kernel_spmd(nc, ins, core_ids=list(range(N_CORES)),
                                   trace=False)
    LAST_EXEC_NS = res.exec_time_ns

    out = np.empty((N, HC), np.float32)
    for k in range(N_CORES):
        node0, n_nodes = meta[k]
        out[node0:node0 + n_nodes] = res.results[k]["out"][:n_nodes]
    out += bias[None, :]
    return out


# revision 6
# speedup vs baseline: 3.9260x; 3.9260x over previous
"""GATv2Conv kernel for 8 Trainium2 NeuronCores.

Strategy: destination-node sharding. Edges (with self loops) are sorted by
destination row and split into 8 contiguous node ranges with balanced edge
counts. Per core the host ships a per-edge stream s_e = x[row_e] + x[col_e]
(transposed, channels on partitions). The device computes, per 128-edge tile:
  E = s @ W            (PE, two layouts: ch-on-part and edge-on-part)
  e_act = lrelu(E)     (ACT)
  alpha_T = e_act^T @ attmask   (PE)  -> ea = exp(alpha_T) (ACT)
  wmsg = E_T * ea      (DVE, broadcast over channels)
  acc += sel^T @ [wmsg | ea]    (PE, selection matrix from is_equal vs iota)
Per 128-node stripe the accumulated numerator is corrected by
  num = acc[:, :64] - h_i * den   (since E = h_i + h_j)
and divided by den = acc[:, 64:68]. h_i = x_slice @ W computed on device.
No softmax max-subtraction: alpha is O(10) so exp is safe in fp32.
"""
import os
import sys
import types

sys.path.insert(0, "/opt/trn_rl_repo")

import numpy as np
import ml_dtypes

BF16 = ml_dtypes.bfloat16
N = 100000
E_RAW = 1600000
IN = 128
H, C = 4, 16
HC = H * C
N_CORES = 8
P = 128
QUAD = 4  # tiles per quad (batched elementwise/matmul group)

_CACHE = {}
LAST_EXEC_NS = None


def _install_axon_ntff_shim():
    if "antenv.axon_hooks" in sys.modules:
        return
    try:
        sys.path.insert(0, "/root/.axon_site/trn_agent_boot")
        import trn_boot  # type: ignore

        hook = trn_boot._ntff_profile_via_ctypes("/opt/axon/libaxon_pjrt.so")
        mod = types.ModuleType("antenv.axon_hooks")
        _state = {"hook": hook}
        mod.set_axon_ntff_profile_hook = lambda h: _state.__setitem__("hook", h)
        mod.get_axon_ntff_profile_hook = lambda: _state["hook"]
        sys.modules["antenv.axon_hooks"] = mod
        import antenv

        antenv.axon_hooks = mod
    except Exception:
        pass


def _build_program(S, TPS):
    from concourse import bass, bacc, mybir
    import concourse.tile as tile

    key = (S, TPS)
    if key in _CACHE:
        return _CACHE[key]

    T = S * TPS
    f32 = mybir.dt.float32
    bf16 = mybir.dt.bfloat16
    nc = bacc.Bacc("TRN2", target_bir_lowering=False, debug=False,
                   num_devices=N_CORES)
    sT = nc.dram_tensor("sT", [P, T * P], bf16, kind="ExternalInput")
    xsT = nc.dram_tensor("xsT", [P, S * P], bf16, kind="ExternalInput")
    rowrel = nc.dram_tensor("rowrel", [P, T], bf16, kind="ExternalInput")
    Wt = nc.dram_tensor("W", [IN, HC], bf16, kind="ExternalInput")
    attm = nc.dram_tensor("attm", [HC, H], bf16, kind="ExternalInput")
    out_d = nc.dram_tensor("out", [S * P, HC], f32, kind="ExternalOutput")

    NQ = TPS // QUAD  # quads per stripe
    assert TPS % QUAD == 0

    with tile.TileContext(nc) as tc:
        with (
            tc.tile_pool(name="const", bufs=1) as constp,
            tc.tile_pool(name="stream", bufs=2) as streamp,
            tc.tile_pool(name="work", bufs=3) as workp,
            tc.tile_pool(name="ep", bufs=2) as epp,
            tc.tile_pool(name="ps_e", bufs=2, space="PSUM") as ps_e,
            tc.tile_pool(name="ps_q", bufs=2, space="PSUM") as ps_q,
            tc.tile_pool(name="ps_at", bufs=1, space="PSUM") as ps_at,
            tc.tile_pool(name="ps_acc", bufs=2, space="PSUM") as ps_acc,
            tc.tile_pool(name="ps_h", bufs=1, space="PSUM") as ps_h,
        ):
            W_sb = constp.tile([IN, HC], bf16, tag="w")
            nc.sync.dma_start(W_sb[:], Wt[:])
            attm_sb = constp.tile([HC, H], bf16, tag="attm")
            nc.sync.dma_start(attm_sb[:], attm[:])
            rr_sb = constp.tile([P, T], bf16, tag="rr")
            nc.sync.dma_start(rr_sb[:], rowrel[:])
            iota_i = constp.tile([P, QUAD * P], mybir.dt.int32, tag="ioti")
            nc.gpsimd.iota(iota_i[:], pattern=[[0, QUAD], [1, P]], base=0,
                           channel_multiplier=0)
            iota_f = constp.tile([P, QUAD * P], bf16, tag="iotf")
            nc.vector.tensor_copy(iota_f[:], iota_i[:])

            for s in range(S):
                stream_sb = streamp.tile([P, TPS * P], bf16, tag="stream")
                nc.sync.dma_start(stream_sb[:], sT[:, s * TPS * P:(s + 1) * TPS * P])
                acc_ps = ps_acc.tile([P, HC + H], f32, tag="acc")
                for q in range(NQ):
                    e_ps = ps_e.tile([HC, QUAD * P], f32, tag="e")
                    # E (ch-on-part) for the whole quad
                    nc.tensor.matmul(
                        out=e_ps[:],
                        lhsT=W_sb[:],
                        rhs=stream_sb[:, q * QUAD * P:(q + 1) * QUAD * P],
                        start=True, stop=True)
                    # lrelu(x) with slope 0.2 = 0.4*(1.5x + |x|); the 0.4 is
                    # folded into attmask on the host.
                    e_abs = workp.tile([HC, QUAD * P], f32, tag="eabs")
                    nc.scalar.activation(
                        out=e_abs[:], in_=e_ps[:],
                        func=mybir.ActivationFunctionType.Abs)
                    e_act = workp.tile([HC, QUAD * P], bf16, tag="eact")
                    nc.vector.scalar_tensor_tensor(
                        out=e_act[:], in0=e_ps[:], scalar=1.5,
                        in1=e_abs[:], op0=mybir.AluOpType.mult,
                        op1=mybir.AluOpType.add)
                    q_ps = ps_q.tile([P, QUAD, HC + H], f32, tag="q")
                    at_ps = ps_at.tile([P, QUAD, H], f32, tag="at")
                    wmsg = workp.tile([P, QUAD, HC + H], bf16, tag="wmsg")
                    sel = workp.tile([P, QUAD * P], bf16, tag="sel")
                    nc.vector.tensor_tensor(
                        out=sel[:].rearrange("p (q n) -> p q n", q=QUAD),
                        in0=rr_sb[:, s * TPS + q * QUAD:s * TPS + (q + 1) * QUAD]
                            .rearrange("p (q o) -> p q o", o=1)
                            .to_broadcast([P, QUAD, P]),
                        in1=iota_f[:].rearrange("p (q n) -> p q n", q=QUAD),
                        op=mybir.AluOpType.is_equal)
                    for i in range(QUAD):
                        t = q * QUAD + i
                        # E_T (edge-on-part)
                        nc.tensor.matmul(
                            out=q_ps[:, i, 0:HC],
                            lhsT=stream_sb[:, t * P:(t + 1) * P],
                            rhs=W_sb[:], start=True, stop=True)
                        # alpha_T
                        nc.tensor.matmul(
                            out=at_ps[:, i, :],
                            lhsT=e_act[:, i * P:(i + 1) * P],
                            rhs=attm_sb[:], start=True, stop=True)
                    # ea = exp(alpha) for the quad, written into wmsg tail
                    nc.scalar.activation(
                        out=wmsg[:, :, HC:HC + H], in_=at_ps[:],
                        func=mybir.ActivationFunctionType.Exp)
                    # wmsg head = E_T * ea (broadcast over 16 channels)
                    nc.vector.tensor_tensor(
                        out=wmsg[:, :, 0:HC].rearrange("p q (h c) -> p q h c", h=H),
                        in0=q_ps[:, :, 0:HC].rearrange("p q (h c) -> p q h c", h=H),
                        in1=wmsg[:, :, HC:HC + H].to_broadcast([P, QUAD, H, C]),
                        op=mybir.AluOpType.mult)
                    for i in range(QUAD):
                        nc.tensor.matmul(
                            out=acc_ps[:],
                            lhsT=sel[:, i * P:(i + 1) * P],
                            rhs=wmsg[:, i, :],
                            start=(q == 0 and i == 0),
                            stop=(q == NQ - 1 and i == QUAD - 1))
                # epilogue
                xs_sb = epp.tile([P, P], bf16, tag="xs")
                nc.sync.dma_start(xs_sb[:], xsT[:, s * P:(s + 1) * P])
                h_ps = ps_h.tile([P, HC], f32, tag="h")
                nc.tensor.matmul(
                    out=h_ps[:], lhsT=xs_sb[:],
                    rhs=W_sb[:], start=True, stop=True)
                acc_sb = epp.tile([P, HC + H], f32, tag="accsb")
                nc.scalar.activation(out=acc_sb[:], in_=acc_ps[:],
                                     func=mybir.ActivationFunctionType.Copy)
                h_sb = epp.tile([P, HC], f32, tag="hsb")
                nc.scalar.activation(out=h_sb[:], in_=h_ps[:],
                                     func=mybir.ActivationFunctionType.Copy)
                rec = epp.tile([P, H], f32, tag="rec")
                nc.vector.reciprocal(rec[:], acc_sb[:, HC:HC + H])
                tmp = epp.tile([P, HC], f32, tag="tmp")
                # tmp = h_i * den
                nc.vector.tensor_tensor(
                    out=tmp[:].rearrange("p (h c) -> p h c", h=H),
                    in0=h_sb[:].rearrange("p (h c) -> p h c", h=H),
                    in1=acc_sb[:, HC:HC + H].to_broadcast([P, H, C]),
                    op=mybir.AluOpType.mult)
                # tmp = acc_num - tmp
                nc.vector.tensor_tensor(
                    out=tmp[:], in0=acc_sb[:, 0:HC], in1=tmp[:],
                    op=mybir.AluOpType.subtract)
                out_sb = epp.tile([P, HC], f32, tag="outsb")
                nc.vector.tensor_tensor(
                    out=out_sb[:].rearrange("p (h c) -> p h c", h=H),
                    in0=tmp[:].rearrange("p (h c) -> p h c", h=H),
                    in1=rec[:].to_broadcast([P, H, C]),
                    op=mybir.AluOpType.mult)
                nc.sync.dma_start(out_d[s * P:(s + 1) * P, :], out_sb[:])
    nc.compile()
    _CACHE[key] = nc
    return nc


def _prep(x, edge_index):
    """Returns per-core input maps + (S, TPS, core node offsets/counts)."""
    x = np.asarray(x, dtype=np.float32)
    rows = np.concatenate([np.asarray(edge_index[0]), np.arange(N, dtype=np.int64)])
    cols = np.concatenate([np.asarray(edge_index[1]), np.arange(N, dtype=np.int64)])
    order = np.argsort(rows, kind="stable")
    rows = rows[order]
    cols = cols[order]
    Etot = rows.shape[0]

    n_stripes = (N + P - 1) // P  # 782
    # edges per stripe via searchsorted on sorted rows
    stripe_starts = np.searchsorted(rows, np.arange(n_stripes) * P)
    stripe_ends = np.searchsorted(rows, np.minimum((np.arange(n_stripes) + 1) * P, N))
    stripe_cnt = stripe_ends - stripe_starts
    cum = np.cumsum(stripe_cnt)
    # balanced contiguous stripe ranges per core
    bounds = [0]
    for k in range(1, N_CORES):
        bounds.append(int(np.searchsorted(cum, Etot * k / N_CORES)))
    bounds.append(n_stripes)
    S = max(bounds[k + 1] - bounds[k] for k in range(N_CORES))
    TPS = 0
    for s in range(n_stripes):
        TPS = max(TPS, (int(stripe_cnt[s]) + P - 1) // P)
    TPS = ((TPS + QUAD - 1) // QUAD) * QUAD
    T = S * TPS

    x_ext = np.vstack([x, np.zeros((1, IN), np.float32)])  # pad row -> zeros
    ins = []
    meta = []
    for k in range(N_CORES):
        s0, s1 = bounds[k], bounds[k + 1]
        node0 = s0 * P
        n_nodes = min(s1 * P, N) - node0
        slot_rows = np.full(T * P, N, dtype=np.int64)   # pad -> zero row
        slot_cols = np.full(T * P, N, dtype=np.int64)
        rowrel = np.full(T * P, 999.0, dtype=np.float32)
        for si in range(s1 - s0):
            s = s0 + si
            e0, e1 = int(stripe_starts[s]), int(stripe_ends[s])
            cnt = e1 - e0
            base = si * TPS * P
            slot_rows[base:base + cnt] = rows[e0:e1]
            slot_cols[base:base + cnt] = cols[e0:e1]
            rowrel[base:base + cnt] = (rows[e0:e1] - (node0 + si * P)).astype(np.float32)
        sA = x_ext[slot_rows]
        sA += x_ext[slot_cols]
        sT = np.ascontiguousarray(sA.T.astype(BF16))
        del sA
        # x slice (transposed) for h_i
        sl = np.arange(node0, node0 + S * P)
        sl = np.where(sl < N, sl, N)
        xsT = np.ascontiguousarray(x_ext[sl].T.astype(BF16))
        rr = np.ascontiguousarray(rowrel.reshape(T, P).T.astype(BF16))
        ins.append({"sT": sT, "xsT": xsT, "rowrel": rr})
        meta.append((node0, n_nodes))
    return ins, meta, S, TPS


def kernel(x, edge_index, W, att, bias):
    global LAST_EXEC_NS
    _install_axon_ntff_shim()
    from concourse.bass_utils import run_bass_kernel_spmd

    W = np.asarray(W, dtype=np.float32)
    att = np.asarray(att, dtype=np.float32)
    bias = np.asarray(bias, dtype=np.float32)

    ins, meta, S, TPS = _prep(x, edge_index)
    attm = np.zeros((HC, H), np.float32)
    attf = att[0].reshape(HC)  # (h, c) flattened, h-major
    for hc in range(HC):
        # 0.4 factor from lrelu decomposition 0.2-slope = 0.4*(1.5x + |x|)
        attm[hc, hc // C] = 0.4 * attf[hc]
    W16 = W.astype(BF16)
    attm16 = attm.astype(BF16)
    for m in ins:
        m["W"] = W16
        m["attm"] = attm16

    nc = _build_program(S, TPS)
    trace = os.environ.get("KERNEL_TRACE", "1") == "1"
    try:
        res = run_bass_kernel_spmd(nc, ins, core_ids=list(range(N_CORES)),
                                   trace=trace)
    except Exception:
        if not trace:
            raise
        res = run_bass_kernel_spmd(nc, ins, core_ids=list(range(N_CORES)),
                                   trace=False)
    LAST_EXEC_NS = res.exec_time_ns

    out = np.empty((N, HC), np.float32)
    for k in range(N_CORES):
        node0, n_nodes = meta[k]
        out[node0:node0 + n_nodes] = res.results[k]["out"][:n_nodes]
    out += bias[None, :]
    return out


# revision 7
# speedup vs baseline: 4.8199x; 1.2277x over previous
"""GATv2Conv kernel for 8 Trainium2 NeuronCores.

Strategy: destination-node sharding. Edges (with self loops) are sorted by
destination row and split into 8 contiguous node ranges with balanced edge
counts. Per core the host ships a per-edge stream s_e = x[row_e] + x[col_e]
(transposed, channels on partitions). The device computes, per 128-edge tile:
  E = s @ W            (PE, two layouts: ch-on-part and edge-on-part)
  e_act = lrelu(E)     (ACT)
  alpha_T = e_act^T @ attmask   (PE)  -> ea = exp(alpha_T) (ACT)
  wmsg = E_T * ea      (DVE, broadcast over channels)
  acc += sel^T @ [wmsg | ea]    (PE, selection matrix from is_equal vs iota)
Per 128-node stripe the accumulated numerator is corrected by
  num = acc[:, :64] - h_i * den   (since E = h_i + h_j)
and divided by den = acc[:, 64:68]. h_i = x_slice @ W computed on device.
No softmax max-subtraction: alpha is O(10) so exp is safe in fp32.
"""
import os
import sys
import types

sys.path.insert(0, "/opt/trn_rl_repo")

import numpy as np
import ml_dtypes

BF16 = ml_dtypes.bfloat16
N = 100000
E_RAW = 1600000
IN = 128
H, C = 4, 16
HC = H * C
N_CORES = 8
P = 128
QUAD = 4  # tiles per quad (batched elementwise/matmul group)

_CACHE = {}
LAST_EXEC_NS = None


def _install_axon_ntff_shim():
    if "antenv.axon_hooks" in sys.modules:
        return
    try:
        sys.path.insert(0, "/root/.axon_site/trn_agent_boot")
        import trn_boot  # type: ignore

        hook = trn_boot._ntff_profile_via_ctypes("/opt/axon/libaxon_pjrt.so")
        mod = types.ModuleType("antenv.axon_hooks")
        _state = {"hook": hook}
        mod.set_axon_ntff_profile_hook = lambda h: _state.__setitem__("hook", h)
        mod.get_axon_ntff_profile_hook = lambda: _state["hook"]
        sys.modules["antenv.axon_hooks"] = mod
        import antenv

        antenv.axon_hooks = mod
    except Exception:
        pass


def _build_program(S, TPS):
    from concourse import bass, bacc, mybir
    import concourse.tile as tile

    key = (S, TPS)
    if key in _CACHE:
        return _CACHE[key]

    T = S * TPS
    f32 = mybir.dt.float32
    bf16 = mybir.dt.bfloat16
    nc = bacc.Bacc("TRN2", target_bir_lowering=False, debug=False,
                   num_devices=N_CORES)
    sT = nc.dram_tensor("sT", [P, T * P], bf16, kind="ExternalInput")
    xsT = nc.dram_tensor("xsT", [P, S * P], bf16, kind="ExternalInput")
    rowrel = nc.dram_tensor("rowrel", [P, T], bf16, kind="ExternalInput")
    Wt = nc.dram_tensor("W", [IN, HC], bf16, kind="ExternalInput")
    attr = nc.dram_tensor("attr", [P, QUAD * HC], bf16, kind="ExternalInput")
    out_d = nc.dram_tensor("out", [S * P, HC], f32, kind="ExternalOutput")

    NQ = TPS // QUAD  # quads per stripe
    assert TPS % QUAD == 0

    with tile.TileContext(nc) as tc:
        with (
            tc.tile_pool(name="const", bufs=1) as constp,
            tc.tile_pool(name="stream", bufs=2) as streamp,
            tc.tile_pool(name="work", bufs=3) as workp,
            tc.tile_pool(name="ep", bufs=2) as epp,
            tc.tile_pool(name="ps_q", bufs=3, space="PSUM") as ps_q,
            tc.tile_pool(name="ps_acc", bufs=2, space="PSUM") as ps_acc,
            tc.tile_pool(name="ps_h", bufs=2, space="PSUM") as ps_h,
        ):
            W_sb = constp.tile([IN, HC], bf16, tag="w")
            nc.sync.dma_start(W_sb[:], Wt[:])
            attr_sb = constp.tile([P, QUAD * HC], bf16, tag="attr")
            nc.sync.dma_start(attr_sb[:], attr[:])
            rr_sb = constp.tile([P, T], bf16, tag="rr")
            nc.sync.dma_start(rr_sb[:], rowrel[:])
            iota_i = constp.tile([P, QUAD * P], mybir.dt.int32, tag="ioti")
            nc.gpsimd.iota(iota_i[:], pattern=[[0, QUAD], [1, P]], base=0,
                           channel_multiplier=0)
            iota_f = constp.tile([P, QUAD * P], bf16, tag="iotf")
            nc.vector.tensor_copy(iota_f[:], iota_i[:])

            for s in range(S):
                stream_sb = streamp.tile([P, TPS * P], bf16, tag="stream")
                nc.sync.dma_start(stream_sb[:], sT[:, s * TPS * P:(s + 1) * TPS * P])
                acc_ps = ps_acc.tile([P, HC + H], f32, tag="acc")
                for q in range(NQ):
                    q_ps = ps_q.tile([P, QUAD, HC], f32, tag="q")
                    for i in range(QUAD):
                        t = q * QUAD + i
                        # E_T (edge-on-part)
                        nc.tensor.matmul(
                            out=q_ps[:, i, :],
                            lhsT=stream_sb[:, t * P:(t + 1) * P],
                            rhs=W_sb[:], start=True, stop=True)
                    # E_T copy to SBUF (bf16) for DVE/GPSIMD consumers
                    q_sb = workp.tile([P, QUAD * HC], bf16, tag="qsb")
                    nc.scalar.activation(
                        out=q_sb[:], in_=q_ps[:].rearrange("p q c -> p (q c)"),
                        func=mybir.ActivationFunctionType.Copy)
                    # lrelu via parametric relu on ACT
                    e_act = workp.tile([P, QUAD * HC], bf16, tag="eact")
                    nc.scalar.activation(
                        out=e_act[:], in_=q_ps[:].rearrange("p q c -> p (q c)"),
                        func=mybir.ActivationFunctionType.Prelu, alpha=0.2)
                    # alpha pre-products on gpsimd: e_act * att (per channel)
                    prod = workp.tile([P, QUAD * HC], bf16, tag="prod")
                    nc.gpsimd.tensor_tensor(
                        out=prod[:], in0=e_act[:], in1=attr_sb[:],
                        op=mybir.AluOpType.mult)
                    # alpha = segmented reduce over the 16 channels per head
                    at_sb = workp.tile([P, QUAD * H], f32, tag="at")
                    nc.vector.tensor_reduce(
                        out=at_sb[:].rearrange("p (q h) -> p q h", q=QUAD),
                        in_=prod[:].rearrange("p (q h c) -> p q h c", q=QUAD, h=H),
                        axis=mybir.AxisListType.X,
                        op=mybir.AluOpType.add)
                    wmsg = workp.tile([P, QUAD, HC + H], bf16, tag="wmsg")
                    # ea = exp(alpha), written into wmsg tail
                    nc.scalar.activation(
                        out=wmsg[:, :, HC:HC + H],
                        in_=at_sb[:].rearrange("p (q h) -> p q h", q=QUAD),
                        func=mybir.ActivationFunctionType.Exp)
                    # wmsg head = E_T * ea (broadcast over 16 channels)
                    nc.vector.tensor_tensor(
                        out=wmsg[:, :, 0:HC].rearrange("p q (h c) -> p q h c", h=H),
                        in0=q_sb[:].rearrange("p (q h c) -> p q h c", q=QUAD, h=H),
                        in1=wmsg[:, :, HC:HC + H].to_broadcast([P, QUAD, H, C]),
                        op=mybir.AluOpType.mult)
                    sel = workp.tile([P, QUAD * P], bf16, tag="sel")
                    nc.vector.tensor_tensor(
                        out=sel[:].rearrange("p (q n) -> p q n", q=QUAD),
                        in0=rr_sb[:, s * TPS + q * QUAD:s * TPS + (q + 1) * QUAD]
                            .rearrange("p (q o) -> p q o", o=1)
                            .to_broadcast([P, QUAD, P]),
                        in1=iota_f[:].rearrange("p (q n) -> p q n", q=QUAD),
                        op=mybir.AluOpType.is_equal)
                    for i in range(QUAD):
                        nc.tensor.matmul(
                            out=acc_ps[:],
                            lhsT=sel[:, i * P:(i + 1) * P],
                            rhs=wmsg[:, i, :],
                            start=(q == 0 and i == 0),
                            stop=(q == NQ - 1 and i == QUAD - 1))
                # epilogue
                xs_sb = epp.tile([P, P], bf16, tag="xs")
                nc.sync.dma_start(xs_sb[:], xsT[:, s * P:(s + 1) * P])
                h_ps = ps_h.tile([P, HC], f32, tag="h")
                nc.tensor.matmul(
                    out=h_ps[:], lhsT=xs_sb[:],
                    rhs=W_sb[:], start=True, stop=True)
                acc_sb = epp.tile([P, HC + H], f32, tag="accsb")
                nc.scalar.activation(out=acc_sb[:], in_=acc_ps[:],
                                     func=mybir.ActivationFunctionType.Copy)
                h_sb = epp.tile([P, HC], f32, tag="hsb")
                nc.scalar.activation(out=h_sb[:], in_=h_ps[:],
                                     func=mybir.ActivationFunctionType.Copy)
                rec = epp.tile([P, H], f32, tag="rec")
                nc.vector.reciprocal(rec[:], acc_sb[:, HC:HC + H])
                tmp = epp.tile([P, HC], f32, tag="tmp")
                # tmp = h_i * den
                nc.vector.tensor_tensor(
                    out=tmp[:].rearrange("p (h c) -> p h c", h=H),
                    in0=h_sb[:].rearrange("p (h c) -> p h c", h=H),
                    in1=acc_sb[:, HC:HC + H].to_broadcast([P, H, C]),
                    op=mybir.AluOpType.mult)
                # tmp = acc_num - tmp
                nc.vector.tensor_tensor(
                    out=tmp[:], in0=acc_sb[:, 0:HC], in1=tmp[:],
                    op=mybir.AluOpType.subtract)
                out_sb = epp.tile([P, HC], f32, tag="outsb")
                nc.vector.tensor_tensor(
                    out=out_sb[:].rearrange("p (h c) -> p h c", h=H),
                    in0=tmp[:].rearrange("p (h c) -> p h c", h=H),
                    in1=rec[:].to_broadcast([P, H, C]),
                    op=mybir.AluOpType.mult)
                nc.sync.dma_start(out_d[s * P:(s + 1) * P, :], out_sb[:])
    nc.compile()
    _CACHE[key] = nc
    return nc


def _prep(x, edge_index):
    """Returns per-core input maps + (S, TPS, core node offsets/counts)."""
    x = np.asarray(x, dtype=np.float32)
    rows = np.concatenate([np.asarray(edge_index[0]), np.arange(N, dtype=np.int64)])
    cols = np.concatenate([np.asarray(edge_index[1]), np.arange(N, dtype=np.int64)])
    order = np.argsort(rows, kind="stable")
    rows = rows[order]
    cols = cols[order]
    Etot = rows.shape[0]

    n_stripes = (N + P - 1) // P  # 782
    # edges per stripe via searchsorted on sorted rows
    stripe_starts = np.searchsorted(rows, np.arange(n_stripes) * P)
    stripe_ends = np.searchsorted(rows, np.minimum((np.arange(n_stripes) + 1) * P, N))
    stripe_cnt = stripe_ends - stripe_starts
    cum = np.cumsum(stripe_cnt)
    # balanced contiguous stripe ranges per core
    bounds = [0]
    for k in range(1, N_CORES):
        bounds.append(int(np.searchsorted(cum, Etot * k / N_CORES)))
    bounds.append(n_stripes)
    S = max(bounds[k + 1] - bounds[k] for k in range(N_CORES))
    TPS = 0
    for s in range(n_stripes):
        TPS = max(TPS, (int(stripe_cnt[s]) + P - 1) // P)
    TPS = ((TPS + QUAD - 1) // QUAD) * QUAD
    T = S * TPS

    x_ext = np.vstack([x, np.zeros((1, IN), np.float32)])  # pad row -> zeros
    ins = []
    meta = []
    for k in range(N_CORES):
        s0, s1 = bounds[k], bounds[k + 1]
        node0 = s0 * P
        n_nodes = min(s1 * P, N) - node0
        slot_rows = np.full(T * P, N, dtype=np.int64)   # pad -> zero row
        slot_cols = np.full(T * P, N, dtype=np.int64)
        rowrel = np.full(T * P, 999.0, dtype=np.float32)
        for si in range(s1 - s0):
            s = s0 + si
            e0, e1 = int(stripe_starts[s]), int(stripe_ends[s])
            cnt = e1 - e0
            base = si * TPS * P
            slot_rows[base:base + cnt] = rows[e0:e1]
            slot_cols[base:base + cnt] = cols[e0:e1]
            rowrel[base:base + cnt] = (rows[e0:e1] - (node0 + si * P)).astype(np.float32)
        sA = x_ext[slot_rows]
        sA += x_ext[slot_cols]
        sT = np.ascontiguousarray(sA.T.astype(BF16))
        del sA
        # x slice (transposed) for h_i
        sl = np.arange(node0, node0 + S * P)
        sl = np.where(sl < N, sl, N)
        xsT = np.ascontiguousarray(x_ext[sl].T.astype(BF16))
        rr = np.ascontiguousarray(rowrel.reshape(T, P).T.astype(BF16))
        ins.append({"sT": sT, "xsT": xsT, "rowrel": rr})
        meta.append((node0, n_nodes))
    return ins, meta, S, TPS


def kernel(x, edge_index, W, att, bias):
    global LAST_EXEC_NS
    _install_axon_ntff_shim()
    from concourse.bass_utils import run_bass_kernel_spmd

    W = np.asarray(W, dtype=np.float32)
    att = np.asarray(att, dtype=np.float32)
    bias = np.asarray(bias, dtype=np.float32)

    ins, meta, S, TPS = _prep(x, edge_index)
    attf = att[0].reshape(HC)  # (h, c) flattened, h-major
    attr = np.tile(attf[None, :], (P, QUAD)).astype(BF16)
    W16 = W.astype(BF16)
    for m in ins:
        m["W"] = W16
        m["attr"] = attr

    nc = _build_program(S, TPS)
    trace = os.environ.get("KERNEL_TRACE", "1") == "1"
    try:
        res = run_bass_kernel_spmd(nc, ins, core_ids=list(range(N_CORES)),
                                   trace=trace)
    except Exception:
        if not trace:
            raise
        res = run_bass_kernel_spmd(nc, ins, core_ids=list(range(N_CORES)),
                                   trace=False)
    LAST_EXEC_NS = res.exec_time_ns

    out = np.empty((N, HC), np.float32)
    for k in range(N_CORES):
        node0, n_nodes = meta[k]
        out[node0:node0 + n_nodes] = res.results[k]["out"][:n_nodes]
    out += bias[None, :]
    return out
